# revision 1
# baseline (speedup 1.0000x reference)
"""Trainium2 Bass kernel for causal multi-head attention with RoPE.

Problem: B=4, S=2048, D=1024, H=16, DK=64 dense transformer attention
(q/k/v projections -> interleaved RoPE on q,k -> causal softmax attention
-> output projection), fp32 inputs/outputs.

Sharding: 8 NeuronCores, core c handles batch b=c//2 and head-group
g=c%2 (8 of the 16 heads).  Each core computes a partial o_proj output
for its batch over its heads; the host sums the two partials per batch.

Kernel design (per core) — bf16 data path, Act-engine-bound pipeline:
  - All matmul operands bf16 (HW-measured ~0.62 cyc/row, slightly faster
    than f32r; fp8-DoubleRow measured 2.3x SLOWER on HW, so not used).
    Host delivers x/W in bf16 kk-chunk-major layouts; PSUM stays fp32.
  - RoPE in bf16 on DVE+Pool using host cos/sin tables; results
    DMA-permuted into head-contiguous bf16 qrh/krh tiles.
  - scores: S_ps[kv, q] = k_chunk @ qT, two heads per PE pass via
    tile_position row groups (K=64 each), both heads' scores in one
    [128, 1024] PSUM tile (2 banks); ONE merged exp per (hp, chunk)
    -> pt bf16 (the ~400ns fixed Act-instruction overhead dominates,
    so fewer/bigger exps win; the exp stream is the kernel's
    bottleneck engine at ~1.04 ns/col).
  - causal mask applied POST-exp as a 0/1 bf16 multiply on the Pool
    engine (gpsimd cannot touch PSUM; this keeps DVE/Act free).
  - v stored bf16 with a per-head ones-column (65 cols/head) so attn@v
    also produces the softmax denominator row; attn@v in bf16.
  - software pipelining: scores/exp of chunk c+1 issue before attn@v of
    chunk c, so the in-order PE queue never parks waiting on the exp.
  - projection work for block sc+1 is interleaved INTO attention(qg=sc)
    at head-pair boundaries (fill slots), so the Act engine always has
    exp work queued while the PE runs projections.
  - normalize: broadcast the denominator row l across 64 partitions with
    a K=1 ones matmul into a shared scratch PSUM ring, reciprocal +
    multiply on DVE -> normalized bf16 outT per head.  Each hp's
    normalize is EMISSION-DEFERRED until after chunk 1 of the next hp
    (but before its first attn@v, preserving the WAR order on the O
    ring), so the broadcast matmuls no longer sit at the head of the PE
    queue waiting on DVE and starving the Act engine at hp boundaries.
  - o_proj in bf16 accumulated in PSUM; outputs DMA'd as bf16, host sums
    the two per-batch partials in fp32.  o_proj(qg) emission is DEFERRED
    into attention(qg+1)'s chunk loop (the otn pair ring is 8 deep = two
    qg generations, so the WAR order holds), removing the PE stall on
    the just-emitted normalize at every q-group transition.
  - PSUM budget (8 banks): shared scratch ring (proj/o_proj/rbp) 2 +
    scores 2x2 + O accumulators 2.
  - DMA routing: HWDGE queues (SP/Act) for bulk loads/permutes/stores,
    with producers' results consumed from the same queue to avoid
    head-of-line blocking; x tiles prefetched one block ahead.
"""

import sys

sys.path.insert(0, "/opt/trn_rl_repo")

from contextlib import ExitStack

import numpy as np

import concourse.bass as bass
import concourse.tile as tile
from concourse import bacc, mybir
from concourse.bass_utils import run_bass_kernel_spmd

B, S, D, H = 4, 2048, 1024, 16
DK = D // H          # 64
NHL = 8              # heads per core (local)
QR = NHL * DK        # 512 projected rows per core
NKC = S // 128       # 16 kv chunks
THETA = 10000.0

F32 = mybir.dt.float32
BF16 = mybir.dt.bfloat16

_COMPILED = None


def build_kernel(reps=1):
    nc = bacc.Bacc("TRN2", target_bir_lowering=False, debug=False,
                   enable_asserts=False)

    x8d = nc.dram_tensor("x8", [128, 4 * 8 * 512], BF16, kind="ExternalInput").ap()
    wq8 = nc.dram_tensor("wq8", [128, 8 * QR], BF16, kind="ExternalInput").ap()
    wk8 = nc.dram_tensor("wk8", [128, 8 * QR], BF16, kind="ExternalInput").ap()
    wv8 = nc.dram_tensor("wv8", [128, 8 * QR], BF16, kind="ExternalInput").ap()
    wod = nc.dram_tensor("wod", [128, 4 * D], BF16, kind="ExternalInput").ap()
    cos4 = nc.dram_tensor("cos4", [128, S], BF16, kind="ExternalInput").ap()
    sin4 = nc.dram_tensor("sin4", [128, S], BF16, kind="ExternalInput").ap()
    maskd = nc.dram_tensor("maskd", [128, 256], BF16, kind="ExternalInput").ap()
    out = nc.dram_tensor("out", [S, D], BF16, kind="ExternalOutput").ap()

    with tile.TileContext(nc) as tc, ExitStack() as ctx:
        persist = ctx.enter_context(tc.tile_pool(name="persist", bufs=1))
        # head-contiguous rope'd q/k: chunk hp holds heads (2hp, 2hp+1);
        # within a head: [even-lane j 0..31 ; odd-lane j 0..31]
        qrh = [persist.tile([128, S], BF16, tag=f"qrh{i}", name=f"qrh{i}")
               for i in range(4)]
        krh = [persist.tile([128, S], BF16, tag=f"krh{i}", name=f"krh{i}")
               for i in range(4)]
        # v natural layout, 65 cols per head (64 v + ones), all 16 s-tiles
        v_all = persist.tile([128, NKC * NHL * 65], BF16, tag="v_all")
        vsb = [v_all[:, i * NHL * 65:(i + 1) * NHL * 65] for i in range(NKC)]
        maskt = persist.tile([128, 256], BF16, tag="maskt")
        onest = persist.tile([65, 64], BF16, tag="onest")
        cost_all = persist.tile([128, S], BF16, tag="cost")
        sint_all = persist.tile([128, S], BF16, tag="sint")
        wq = persist.tile([128, 8 * QR], BF16, tag="wq")
        wk = persist.tile([128, 8 * QR], BF16, tag="wk")
        wv = persist.tile([128, 8 * QR], BF16, tag="wv")
        woh = persist.tile([128, 4 * D], BF16, tag="woh")

        nc.sync.dma_start(wq[:], wq8[:])
        nc.scalar.dma_start(maskt[:], maskd[:])
        m3 = maskt[:].rearrange("p (two n) -> p two n", two=2)
        nc.scalar.dma_start(woh[:], wod[:])
        nc.vector.memset(onest[:], 1.0)
        # ones column (col 64 of each head's 65-col block), all kv tiles
        v3 = v_all[:].rearrange("p (n c) -> p n c", c=65)
        nc.gpsimd.memset(v3[:, :, 64:65], 1.0)

        wq3 = wq[:].rearrange("p (k q) -> p k q", k=8)
        wk3 = wk[:].rearrange("p (k q) -> p k q", k=8)
        wv3 = wv[:].rearrange("p (k q) -> p k q", k=8)

        xpool = ctx.enter_context(tc.tile_pool(name="xp", bufs=3))
        stg = ctx.enter_context(tc.tile_pool(name="stg", bufs=2))
        ppool = ctx.enter_context(tc.tile_pool(name="pt", bufs=6))
        otn = ctx.enter_context(tc.tile_pool(name="otn", bufs=8))
        # PSUM budget (8 banks): scratch 2 + sp 2x2 + O 2
        ps_x = ctx.enter_context(
            tc.tile_pool(name="ps_x", bufs=2, space="PSUM"))
        ps_s = ctx.enter_context(
            tc.tile_pool(name="ps_s", bufs=2, space="PSUM"))
        ps_o = ctx.enter_context(
            tc.tile_pool(name="ps_o", bufs=1, space="PSUM"))

        def proj_chunk(w3, xt3, m, names):
            """One m-chunk [128, 512] of a q/k projection, bf16."""
            ps = ps_x.tile([128, 512], F32, tag="scr", name="pps")
            for t in range(8):
                nc.tensor.matmul(
                    ps[:],
                    w3[:, t, m * 128:(m + 1) * 128],
                    xt3[:, t, :],
                    start=(t == 0), stop=(t == 7))
            qs = stg.tile([128, 512], BF16, tag="qps", bufs=12,
                          name=f"{names}{m}")
            nc.vector.tensor_copy(qs[:], ps[:])
            return qs

        def project(w3, xt3, names):
            return [proj_chunk(w3, xt3, m, names) for m in range(4)]

        def vproj_chunk(xt3, sc, st):
            vp = ps_x.tile([128, 512], F32, tag="scr", name="vp")
            for t in range(8):
                nc.tensor.matmul(
                    vp[:],
                    xt3[:, t, st * 128:(st + 1) * 128],
                    wv3[:, t, :],
                    start=(t == 0), stop=(t == 7))
            vdst = vsb[sc * 4 + st][:].rearrange(
                "p (h c) -> p h c", c=65)[:, :, 0:64]
            vsrc = vp[:].rearrange("p (h c) -> p h c", c=64)
            nc.vector.tensor_copy(vdst, vsrc)

        def rope(sb, dst, s0, qk):
            # chunks (0,2) even/odd of heads 0-3, (1,3) heads 4-7
            costc = cost_all[:, s0:s0 + 512]
            sintc = sint_all[:, s0:s0 + 512]
            for me, mo in ((0, 2), (1, 3)):
                hbase = 0 if me == 0 else 4
                te = stg.tile([128, 512], BF16, tag="tmp", bufs=4)
                to = stg.tile([128, 512], BF16, tag="tmp", bufs=4)
                nc.vector.tensor_mul(te[:], sb[me][:], costc)
                nc.gpsimd.tensor_mul(to[:], sb[mo][:], sintc)
                qre = stg.tile([128, 512], BF16, tag="qr", bufs=4)
                nc.vector.tensor_sub(qre[:], te[:], to[:])
                te2 = stg.tile([128, 512], BF16, tag="tmp", bufs=4)
                to2 = stg.tile([128, 512], BF16, tag="tmp", bufs=4)
                nc.gpsimd.tensor_mul(te2[:], sb[mo][:], costc)
                nc.vector.tensor_mul(to2[:], sb[me][:], sintc)
                qro = stg.tile([128, 512], BF16, tag="qr", bufs=4)
                nc.vector.tensor_add(qro[:], te2[:], to2[:])
                # permute into head-contiguous chunks via DMA
                for hl in range(4):
                    h = hbase + hl
                    hp, h01 = h // 2, h % 2
                    nc.sync.dma_start(
                        dst[hp][64 * h01: 64 * h01 + 32, s0:s0 + 512],
                        qre[32 * hl: 32 * hl + 32, :])
                    nc.sync.dma_start(
                        dst[hp][64 * h01 + 32: 64 * h01 + 64, s0:s0 + 512],
                        qro[32 * hl: 32 * hl + 32, :])

        def attention(qg, fill=None, prefill=()):
            q0 = qg * 512
            nchunks = 4 * qg + 4
            otn_tiles = [None] * 4
            prev_norm = None      # deferred normalize of the previous hp
            pending = list(prefill)  # deferred thunks (prev o_proj + proj)

            def make_norm(hp, O):
                def norm():
                    pair = otn.tile([128, 512], BF16, tag="pair", bufs=8,
                                    name="pair")
                    for h01 in range(2):
                        lsb = stg.tile([65, 512], BF16, tag="lsb", bufs=2)
                        nc.vector.tensor_copy(lsb[64:65, :],
                                              O[h01][64:65, :])
                        rbp = ps_x.tile([128, 512], F32, tag="scr",
                                        name="rbp")
                        nc.tensor.matmul(rbp[0:64, :],
                                         onest[64:65, 0:64],
                                         lsb[64:65, :],
                                         start=True, stop=True)
                        rlb = stg.tile([64, 512], F32, tag="rlb", bufs=2)
                        nc.vector.reciprocal(rlb[:], rbp[0:64, :])
                        if h01 == 0:
                            nc.vector.tensor_mul(pair[0:64, :],
                                                 O[h01][0:64, :], rlb[:])
                        else:
                            ot = stg.tile([64, 512], BF16, tag="ot", bufs=2)
                            nc.vector.tensor_mul(ot[:], O[h01][0:64, :],
                                                 rlb[:])
                            nc.sync.dma_start(pair[64:128, :], ot[:])
                    otn_tiles[hp] = pair
                return norm

            for hp in range(4):
                O = [ps_o.tile([65, 512], F32, tag=f"O{h01}", name="O")
                     for h01 in range(2)]
                pend = None

                def emit_av(ent, O=O, hp=hp):
                    c, pt, qoff, N = ent
                    for h01 in range(2):
                        h = 2 * hp + h01
                        nc.tensor.matmul(
                            O[h01][:, qoff:qoff + N],
                            vsb[c][:, 65 * h: 65 * h + 65],
                            pt[:, 512 * h01: 512 * h01 + N],
                            start=(c == 0), stop=(c == nchunks - 1))

                for c in range(nchunks):
                    cmod = c - 4 * qg
                    qoff = 128 * cmod if cmod >= 0 else 0
                    N = 512 - qoff
                    sp = ps_s.tile([128, 1024], F32, tag="S")
                    for h01 in range(2):
                        base = 64 * h01
                        nc.tensor.matmul(
                            sp[:, 512 * h01: 512 * h01 + N],
                            krh[hp][base:base + 64, c * 128:(c + 1) * 128],
                            qrh[hp][base:base + 64, q0 + qoff:q0 + qoff + N],
                            start=True, stop=True,
                            tile_position=(base, 0))
                    pt = ppool.tile([128, 1024], BF16, tag="pt", bufs=8)
                    if N == 512:
                        nc.scalar.activation(
                            pt[:], sp[:],
                            mybir.ActivationFunctionType.Exp, scale=0.125)
                    else:
                        sp3 = sp[:].rearrange("p (two n) -> p two n", two=2)
                        pt3 = pt[:].rearrange("p (two n) -> p two n", two=2)
                        nc.scalar.activation(
                            pt3[:, :, 0:N], sp3[:, :, 0:N],
                            mybir.ActivationFunctionType.Exp, scale=0.125)
                    if cmod >= 0:
                        # causal mask: zero upper triangle post-exp
                        pt3 = pt[:].rearrange("p (two n) -> p two n", two=2)
                        nc.gpsimd.tensor_mul(pt3[:, :, 0:128],
                                             pt3[:, :, 0:128], m3[:])
                    # deferred work between scores/exp and attn@v: the
                    # previous hp's normalize MUST be emitted before this
                    # hp's first attn@v (WAR on the O ring); projection
                    # fill units spread across later chunks.
                    if c == 1 and prev_norm is not None:
                        prev_norm()
                        prev_norm = None
                    elif c >= 2 and pending:
                        pending.pop(0)()
                    if pend is not None:
                        emit_av(pend)
                    pend = (c, pt, qoff, N)
                emit_av(pend)

                if prev_norm is not None:
                    prev_norm()
                prev_norm = make_norm(hp, O)
                if fill is not None:
                    pending.extend(fill[hp])

            prev_norm()
            for thunk in pending:
                thunk()

            # o_proj for this q-group, deferred into the next attention
            def oproj_thunk(qt):
                def run():
                    qtile = qg * 4 + qt
                    osb = stg.tile([128, 1024], BF16, tag="osb", bufs=3)
                    for oh in range(2):
                        f = ps_x.tile([128, 512], F32, tag="scr", name="F")
                        for p in range(4):
                            nc.tensor.matmul(
                                f[:],
                                otn_tiles[p][:, qt * 128:(qt + 1) * 128],
                                woh[:, p * D + oh * 512:
                                    p * D + oh * 512 + 512],
                                start=(p == 0), stop=(p == 3))
                        nc.vector.tensor_copy(
                            osb[:, oh * 512:(oh + 1) * 512], f[:])
                    nc.sync.dma_start(
                        out[qtile * 128:(qtile + 1) * 128, :], osb[:])
                return run
            return [oproj_thunk(qt) for qt in range(4)]

        for _rep in range(reps):
            xts = {}

            def load_xt(sc):
                xt = xpool.tile([128, 8 * 512], BF16, tag="xt")
                nc.sync.dma_start(xt[:], x8d[:, sc * 4096:(sc + 1) * 4096])
                xts[sc] = xt[:].rearrange("p (k s) -> p k s", k=8)

            def proj_fill(sc):
                """Projection work for block sc as 4 thunk-lists, consumed
                at the hp boundaries of the preceding attention call."""
                s0 = sc * 512
                xt3 = xts[sc]
                qsb = []
                ksb = []

                def qm(m):
                    return lambda: qsb.append(proj_chunk(wq3, xt3, m, "q"))

                def km(m):
                    return lambda: ksb.append(proj_chunk(wk3, xt3, m, "k"))

                def vu(st):
                    return lambda: vproj_chunk(xt3, sc, st)

                def rq():
                    rope(qsb, qrh, s0, "q")

                def rk():
                    rope(ksb, krh, s0, "k")

                return [
                    [qm(0), qm(1)],
                    [qm(2), qm(3), vu(0)],
                    [rq, vu(1), vu(2), km(0)],
                    [vu(3), km(1), km(2), km(3), rk],
                ]

            load_xt(0)
            load_xt(1)
            if _rep == 0:
                # bulk weights behind the critical wq+xt loads
                nc.sync.dma_start(cost_all[:], cos4[:])
                nc.sync.dma_start(sint_all[:], sin4[:])
                nc.sync.dma_start(wv[:], wv8[:])
                nc.sync.dma_start(wk[:], wk8[:])
            # prologue: full projection of block 0
            s0 = 0
            xt3 = xts[0]
            qsb = project(wq3, xt3, "q")
            for st in range(4):
                vproj_chunk(xt3, 0, st)
            rope(qsb, qrh, s0, "q")
            ksb = project(wk3, xt3, "k")
            load_xt(2)
            rope(ksb, krh, s0, "k")

            carry = []
            for qg in range(4):
                sc = qg + 1
                if sc < 4:
                    if sc == 2:
                        load_xt(3)
                    carry = attention(qg, fill=proj_fill(sc),
                                      prefill=carry)
                else:
                    carry = attention(qg, prefill=carry)
            for thunk in carry:
                thunk()

    nc.compile()
    return nc


def _rope_perm():
    """Row permutation for Wq/Wk per-core slices: 4 chunks of 128 =
    (heads 0-3 even, heads 4-7 even, heads 0-3 odd, heads 4-7 odd)."""
    perm = []
    for half in (0, 1):
        for hblk in range(2):
            for h in range(4 * hblk, 4 * hblk + 4):
                for j in range(32):
                    perm.append(h * 64 + 2 * j + half)
    return np.array(perm)


def _prep_in_maps(x, token_positions, Wq, Wk, Wv, Wo):
    BF = mybir.dt.np(BF16)
    half = DK // 2
    freqs = (1.0 / (THETA ** (2.0 * np.arange(half, dtype=np.float32) / DK)))
    angles = token_positions.astype(np.float32)[:, None] * freqs[None, :]
    cos = np.cos(angles).astype(np.float32).T    # [32, S]
    sin = np.sin(angles).astype(np.float32).T
    cos4 = np.ascontiguousarray(np.tile(cos, (4, 1))).astype(BF)  # [128, S]
    sin4 = np.ascontiguousarray(np.tile(sin, (4, 1))).astype(BF)

    kv_l = np.arange(128)[:, None]
    q_l = np.arange(128)[None, :]
    m1 = (q_l >= kv_l)
    maskd = np.concatenate([m1, m1], axis=1).astype(BF)  # [128, 256] 0/1

    def chunked(wT, nk):
        # [nk*128, M] -> [128, nk*M] kk-chunk-major
        m = wT.shape[1]
        return np.ascontiguousarray(
            wT.reshape(nk, 128, m).transpose(1, 0, 2).reshape(128, nk * m))

    perm = _rope_perm()
    in_maps = []
    for c in range(8):
        b, g = c // 2, c % 2
        rows = slice(g * QR, (g + 1) * QR)
        wq_g = Wq[rows, :][perm, :].T   # [D, QR]
        wk_g = Wk[rows, :][perm, :].T
        wv_g = Wv[rows, :].T
        xT = x[b].T                      # [D, S]
        x8 = np.ascontiguousarray(
            xT.reshape(8, 128, 4, 512).transpose(1, 2, 0, 3)
            .reshape(128, 4 * 8 * 512)).astype(BF)
        woT = Wo[:, rows].T              # [QR, D]
        wod = np.ascontiguousarray(
            woT.reshape(4, 128, D).transpose(1, 0, 2).reshape(128, 4 * D)
        ).astype(BF)
        in_maps.append({
            "x8": x8,
            "wq8": chunked(wq_g, 8).astype(BF),
            "wk8": chunked(wk_g, 8).astype(BF),
            "wv8": chunked(wv_g, 8).astype(BF),
            "wod": wod,
            "cos4": cos4,
            "sin4": sin4,
            "maskd": maskd,
        })
    return in_maps


def kernel(x, token_positions, Wq, Wk, Wv, Wo):
    global _COMPILED
    x = np.asarray(x, dtype=np.float32)
    token_positions = np.asarray(token_positions)
    Wq = np.asarray(Wq, dtype=np.float32)
    Wk = np.asarray(Wk, dtype=np.float32)
    Wv = np.asarray(Wv, dtype=np.float32)
    Wo = np.asarray(Wo, dtype=np.float32)

    if _COMPILED is None:
        _COMPILED = build_kernel()
    nc = _COMPILED

    in_maps = _prep_in_maps(x, token_positions, Wq, Wk, Wv, Wo)
    res = run_bass_kernel_spmd(nc, in_maps, core_ids=list(range(8)))

    out = np.empty((B, S, D), dtype=np.float32)
    for b in range(B):
        out[b] = (res.results[2 * b]["out"].astype(np.float32)
                  + res.results[2 * b + 1]["out"].astype(np.float32))
    return out


def time_device(inputs, n1=32, n2=128, repeats=2):
    """Async-pipelined device timing: enqueue N executions of the sharded
    PJRT call with device-resident inputs, block once.  The marginal
    (T(n2)-T(n1))/(n2-n1) cancels per-dispatch axon overhead and
    approximates per-execution hardware time.  Returns ns."""
    import time

    import jax
    from jax.sharding import Mesh, NamedSharding, PartitionSpec

    try:
        from jax.experimental.shard_map import shard_map
    except ImportError:
        from jax import shard_map

    from concourse import bass2jax

    global _COMPILED
    if _COMPILED is None:
        _COMPILED = build_kernel()
    nc = _COMPILED
    bass2jax.install_neuronx_cc_hook()

    in_maps = _prep_in_maps(
        np.asarray(inputs["x"], np.float32), np.asarray(inputs["token_positions"]),
        np.asarray(inputs["Wq"], np.float32), np.asarray(inputs["Wk"], np.float32),
        np.asarray(inputs["Wv"], np.float32), np.asarray(inputs["Wo"], np.float32))

    partition_name = (nc.partition_id_tensor.name
                      if nc.partition_id_tensor else None)
    in_names, out_names, out_avals, zero_outs = [], [], [], []
    for alloc in nc.m.functions[0].allocations:
        if not isinstance(alloc, mybir.MemoryLocationSet):
            continue
        name = alloc.memorylocations[0].name
        if alloc.kind == "ExternalInput":
            if name != partition_name:
                in_names.append(name)
        elif alloc.kind == "ExternalOutput":
            out_names.append(name)
            shape = tuple(alloc.tensor_shape)
            dtype = mybir.dt.np(alloc.dtype)
            out_avals.append(jax.core.ShapedArray(shape, dtype))
            zero_outs.append(np.zeros(shape, dtype))
    n_params = len(in_names)
    all_in_names = in_names + out_names
    if partition_name is not None:
        all_in_names = all_in_names + [partition_name]

    def _body(*args):
        operands = list(args)
        if partition_name is not None:
            operands.append(bass2jax.partition_id_tensor())
        outs = bass2jax._bass_exec_p.bind(
            *operands,
            out_avals=tuple(out_avals),
            in_names=tuple(all_in_names),
            out_names=tuple(out_names),
            lowering_input_output_aliases=(),
            sim_require_finite=True,
            sim_require_nnan=True,
            nc=nc,
        )
        return tuple(outs)

    n_cores = 8
    devices = jax.devices()[:n_cores]
    mesh = Mesh(np.asarray(devices), ("core",))
    spec = PartitionSpec("core")
    sharded = jax.jit(
        shard_map(_body, mesh=mesh,
                  in_specs=(spec,) * (n_params + len(out_names)),
                  out_specs=(spec,) * len(out_names), check_rep=False))
    sharding = NamedSharding(mesh, spec)
    dev_args = [
        jax.device_put(
            np.concatenate([np.asarray(in_maps[c][nm]) for c in range(n_cores)],
                           axis=0), sharding)
        for nm in in_names
    ] + [
        jax.device_put(
            np.zeros((n_cores * z.shape[0], *z.shape[1:]), z.dtype), sharding)
        for z in zero_outs
    ]

    jax.block_until_ready(sharded(*dev_args))

    def run_batch(n):
        t0 = time.perf_counter()
        outs = None
        for _ in range(n):
            outs = sharded(*dev_args)
        jax.block_until_ready(outs)
        return time.perf_counter() - t0

    best = None
    for _ in range(repeats):
        ta = run_batch(n1)
        tb = run_batch(n2)
        marg = (tb - ta) / (n2 - n1)
        best = marg if best is None else min(best, marg)
    return best * 1e9



# revision 67
# speedup vs baseline: 1.2551x; 1.2551x over previous
"""Trainium2 Bass kernel for causal multi-head attention with RoPE.

Problem: B=4, S=2048, D=1024, H=16, DK=64 dense transformer attention
(q/k/v projections -> interleaved RoPE on q,k -> causal softmax attention
-> output projection), fp32 inputs/outputs.

Sharding: 8 NeuronCores, core c handles batch b=c//2 and head-group
g=c%2 (8 of the 16 heads).  Each core computes a partial o_proj output
for its batch over its heads; the host sums the two partials per batch.

Kernel design (per core) — bf16 data path, Act-engine-bound pipeline:
  - All matmul operands bf16 (HW-measured ~0.62 cyc/row, slightly faster
    than f32r; fp8-DoubleRow measured 2.3x SLOWER on HW, so not used).
    Host delivers x/W in bf16 kk-chunk-major layouts; PSUM stays fp32.
  - RoPE in bf16 directly in head-contiguous layout: W rows are kept in
    natural (head, dk) order, so the interleaved (2j, 2j+1) pair swap is
    a stream_shuffle (XOR-1 mask within 32-partition blocks) on DVE plus
    two muls and an add against repeated-j cos / sign-folded sin tables.
    No DMA permutes; rope output writes the persistent qrh/krh tiles.
  - scores: S_ps[kv, q] = k_chunk @ qT, two heads per PE pass via
    tile_position row groups (K=64 each), both heads' scores in one
    [128, 1024] PSUM tile (2 banks); ONE merged exp per (hp, chunk)
    -> pt bf16 (the ~400ns fixed Act-instruction overhead dominates,
    so fewer/bigger exps win; the exp stream is the kernel's
    bottleneck engine at ~1.04 ns/col).
  - causal mask applied PRE-exp as an extra PE matmul accumulating
    -3e4 * upper_tri into the diagonal 128-col region of the scores
    PSUM (stationary = -3e4*tri, moving = I128); exp underflows the
    masked region to 0, keeping Pool/DVE out of the Act->PE chain.
  - v stored bf16 with a per-head ones-column (65 cols/head) so attn@v
    also produces the softmax denominator row; attn@v in bf16.
  - software pipelining: scores/exp of chunk c+1 issue before attn@v of
    chunk c, so the in-order PE queue never parks waiting on the exp.
  - deadline-FIFO fill scheduling: all projection / o_proj work units
    live in one FIFO of (deadline, thunk); each unit is emitted either
    when its latest-safe position (qg, hp, chunk) arrives (correctness:
    q/k chunk m before attention hp m, v s-chunks before the diagonal
    attn@v, o_proj before the otn ring wraps) or by a rhythm pop every
    3rd chunk slot, pacing ~1.7us proj units against ~1.5us exps so
    neither PE nor Act starves.  HW A/B showed every-2nd-chunk pops
    starve Act (PE:Act busy ratio is ~1.66).
  - slim prologue: only q/k chunk 0 of block 0 are emitted directly
    (latency-fast variant: rope reads the proj PSUM directly and stays
    entirely on DVE); first exp lands ~15us into the kernel instead of
    ~47us.  Startup DMAs are ordered x block 0 > wq/wk chunk 0 + cos/sin
    block 0 > wv > everything else; exp bias is an explicit memset tile
    (a float bias lowers to a const-AP DMA on the Act queue that made
    every exp transitively wait on table loads).
  - normalize: copy the two l rows into one SBUF tile at the hp's end
    (the DVE drains them during the next hp's chunk 0), then DEFERRED at
    chunk 1 of the next hp (before its first attn@v -- WAR on the O
    ring): K=1 ones-matmul broadcast, reciprocal + multiply on DVE ->
    normalized bf16 pair tile per hp.
  - o_proj in bf16 accumulated in PSUM; outputs DMA'd as bf16, host sums
    the two per-batch partials in fp32.  o_proj(qg) units carry
    staggered deadlines across attention(qg+1)'s last hp so they never
    flush as one 9us PE burst.
  - PSUM budget (8 banks): shared scratch ring (proj/o_proj/rbp) 2 +
    scores 2x2 + O accumulators 2.
  - single input blob [128, 37504] (x | wq | wk | wv | wo | cos/sin |
    mask), sliced by AP in-kernel: fewer per-dispatch PJRT buffers.
"""

import sys

sys.path.insert(0, "/opt/trn_rl_repo")

from contextlib import ExitStack

import numpy as np

import concourse.bass as bass
import concourse.tile as tile
from concourse import bacc, mybir
from concourse.bass_utils import run_bass_kernel_spmd

B, S, D, H = 4, 2048, 1024, 16
DK = D // H          # 64
NHL = 8              # heads per core (local)
QR = NHL * DK        # 512 projected rows per core
NKC = S // 128       # 16 kv chunks
THETA = 10000.0

F32 = mybir.dt.float32
BF16 = mybir.dt.bfloat16

_COMPILED = None

BENCH_VARIANTS = [("r6", {"reps": 6}),
                  ("r6_rhythm2", {"reps": 6, "rhythm_n": 2}),
                  ("r6_t1dve", {"reps": 6, "rope_t1_pool": False})]

# stream_shuffle mask: swap partition pairs (2j, 2j+1) within each 32-block
_SWAP_MASK = [i ^ 1 for i in range(32)]


def _chain(f, g):
    def run():
        f()
        g()
    return run


def build_kernel(reps=1, mask_mm=True, mask_merge=True, rope_t1_pool=True,
                 norm_split=True, rhythm_n=3):
    nc = bacc.Bacc("TRN2", target_bir_lowering=False, debug=False,
                   enable_asserts=False)

    # single input blob [128, 37504]: x8 | wq8 | wk8 | wv8 | wod | cs8 |
    # maskd concatenated along the free dim (fewer per-dispatch buffers).
    # wq8/wk8 are m-chunk-major: [128, m(4) x t(8) x 128] so per-m slices
    # are contiguous 2KB/partition loads; wv8 stays t-major (consumed as
    # full-width moving operands); cs8 = cos|sin interleaved per 512-block
    blob = nc.dram_tensor("blob", [128, 37504], BF16,
                          kind="ExternalInput").ap()
    _off = [0]

    def _slc(n):
        a = _off[0]
        _off[0] += n
        return blob[:, a:a + n]

    x8d = _slc(4 * 8 * 512)
    wq8 = _slc(8 * QR)
    wk8 = _slc(8 * QR)
    wv8 = _slc(8 * QR)
    wod = _slc(4 * D)
    cs8 = _slc(2 * S)
    maskd = _slc(640)
    out = nc.dram_tensor("out", [S, D], BF16, kind="ExternalOutput").ap()

    with tile.TileContext(nc) as tc, ExitStack() as ctx:
        persist = ctx.enter_context(tc.tile_pool(name="persist", bufs=1))
        # head-contiguous rope'd q/k: chunk hp holds heads (2hp, 2hp+1);
        # within a head: [even-lane j 0..31 ; odd-lane j 0..31]
        qrh = [persist.tile([128, S], BF16, tag=f"qrh{i}", name=f"qrh{i}")
               for i in range(4)]
        krh = [persist.tile([128, S], BF16, tag=f"krh{i}", name=f"krh{i}")
               for i in range(4)]
        # v natural layout, 65 cols per head (64 v + ones), all 16 s-tiles
        v_all = persist.tile([128, NKC * NHL * 65], BF16, tag="v_all")
        vsb = [v_all[:, i * NHL * 65:(i + 1) * NHL * 65] for i in range(NKC)]
        maskt = persist.tile([128, 640], BF16, tag="maskt")
        onest = persist.tile([65, 64], BF16, tag="onest")
        cst = persist.tile([128, 2 * S], BF16, tag="cst")
        wq = persist.tile([128, 8 * QR], BF16, tag="wq")
        wk = persist.tile([128, 8 * QR], BF16, tag="wk")
        wv = persist.tile([128, 8 * QR], BF16, tag="wv")
        woh = persist.tile([128, 4 * D], BF16, tag="woh")

        trid = maskt[:, 0:128]
        eye2 = maskt[:, 128:384]
        m3 = maskt[:, 384:640].rearrange("p (two n) -> p two n", two=2)
        nc.vector.memset(onest[:], 1.0)
        # explicit zero bias for exp: a float bias would lower to a const
        # AP DMA'd on the Act queue, making every exp wait on that queue's
        # preceding table loads
        zbias = persist.tile([128, 1], F32, tag="zbias")
        nc.vector.memset(zbias[:], 0.0)
        # ones column (col 64 of each head's 65-col block), all kv tiles
        v3 = v_all[:].rearrange("p (n c) -> p n c", c=65)
        nc.gpsimd.memset(v3[:, :, 64:65], 1.0)

        wq3 = wq[:].rearrange("p (m k c) -> p m k c", m=4, k=8)
        wk3 = wk[:].rearrange("p (m k c) -> p m k c", m=4, k=8)
        wv3 = wv[:].rearrange("p (k q) -> p k q", k=8)

        xpool = ctx.enter_context(tc.tile_pool(name="xp", bufs=3))
        stg = ctx.enter_context(tc.tile_pool(name="stg", bufs=2))
        ppool = ctx.enter_context(tc.tile_pool(name="pt", bufs=6))
        otn = ctx.enter_context(tc.tile_pool(name="otn", bufs=8))
        # PSUM budget (8 banks): scratch 2 + sp 2x2 + O 2
        ps_x = ctx.enter_context(
            tc.tile_pool(name="ps_x", bufs=2, space="PSUM"))
        ps_s = ctx.enter_context(
            tc.tile_pool(name="ps_s", bufs=2, space="PSUM"))
        ps_o = ctx.enter_context(
            tc.tile_pool(name="ps_o", bufs=1, space="PSUM"))

        def proj_chunk(w3, xt3, m, names, raw=False):
            """One m-chunk [128, 512] of a q/k projection, bf16 (raw=True
            returns the f32 PSUM tile without staging to SBUF)."""
            ps = ps_x.tile([128, 512], F32, tag="scr", name="pps")
            for t in range(8):
                nc.tensor.matmul(
                    ps[:],
                    w3[:, m, t, :],
                    xt3[:, t, :],
                    start=(t == 0), stop=(t == 7))
            if raw:
                return ps
            qs = stg.tile([128, 512], BF16, tag="qps", bufs=12,
                          name=f"{names}{m}")
            nc.vector.tensor_copy(qs[:], ps[:])
            return qs

        def vproj_chunk(xt3, sc, st):
            vp = ps_x.tile([128, 512], F32, tag="scr", name="vp")
            for t in range(8):
                nc.tensor.matmul(
                    vp[:],
                    xt3[:, t, st * 128:(st + 1) * 128],
                    wv3[:, t, :],
                    start=(t == 0), stop=(t == 7))
            vdst = vsb[sc * 4 + st][:].rearrange(
                "p (h c) -> p h c", c=65)[:, :, 0:64]
            vsrc = vp[:].rearrange("p (h c) -> p h c", c=64)
            nc.vector.tensor_copy(vdst, vsrc)

        def rope_chunk(sbm, dst, s0, m, fast=False):
            """RoPE one proj m-chunk [128, 512] (heads 2m, 2m+1 in natural
            dk order) straight into dst[m][:, s0:s0+512]: pair-swap via
            stream_shuffle, sign folded into the sin table.  fast=True
            keeps the whole chain on DVE (no cross-engine hops) for
            latency-critical prologue chunks."""
            costc = cst[:, 2 * s0:2 * s0 + 512]
            sintc = cst[:, 2 * s0 + 512:2 * s0 + 1024]
            t2 = stg.tile([128, 512], BF16, tag="tmp", bufs=6)
            pool_mul = (nc.vector if fast else nc.gpsimd).tensor_mul
            pool_mul(t2[:], sbm[:], sintc)
            ss = stg.tile([128, 512], BF16, tag="tmp", bufs=6)
            nc.vector.stream_shuffle(ss[:], t2[:], _SWAP_MASK)
            t1 = stg.tile([128, 512], BF16, tag="tmp", bufs=6)
            if rope_t1_pool and not fast:
                nc.gpsimd.tensor_mul(t1[:], sbm[:], costc)
            else:
                nc.vector.tensor_mul(t1[:], sbm[:], costc)
            nc.vector.tensor_add(dst[m][:, s0:s0 + 512], t1[:], ss[:])

        rhythm = [0]

        def attention(qg, pending):
            """One q-group's attention.  `pending` is a FIFO of
            (deadline, thunk) units: deadline (qg, hp, c) = latest global
            position the unit must be emitted at (correctness); a rhythm
            pop additionally drains one unit every ~3rd chunk so proj PE
            work is paced evenly against the exp stream."""
            q0 = qg * 512
            nchunks = 4 * qg + 4
            otn_tiles = [None] * 4
            prev_norm = None      # deferred normalize of the previous hp

            def flush(pos):
                while pending and pending[0][0] <= pos:
                    pending.pop(0)[1]()

            def make_norm(hp, O):
                """Returns (copy_part, norm): copy_part runs at the hp's
                end (so the DVE drains the l-row copies during the next
                hp's chunk 0), norm is deferred to chunk 1 of the next hp
                (before its first attn@v -- WAR on the O ring)."""
                state = {}

                def copy_part():
                    lsb = stg.tile([65, 1024], BF16, tag="lsb", bufs=2)
                    nc.vector.tensor_copy(lsb[64:65, 0:512],
                                          O[0][64:65, :])
                    nc.vector.tensor_copy(lsb[64:65, 512:1024],
                                          O[1][64:65, :])
                    state["lsb"] = lsb

                def norm():
                    lsb = state["lsb"]
                    pair = otn.tile([128, 512], BF16, tag="pair", bufs=8,
                                    name="pair")
                    for h01 in range(2):
                        rbp = ps_x.tile([128, 512], F32, tag="scr",
                                        name="rbp")
                        nc.tensor.matmul(rbp[0:64, :],
                                         onest[64:65, 0:64],
                                         lsb[64:65,
                                             512 * h01:512 * h01 + 512],
                                         start=True, stop=True)
                        rlb = stg.tile([64, 512], F32, tag="rlb", bufs=2)
                        nc.vector.reciprocal(rlb[:], rbp[0:64, :])
                        if h01 == 0:
                            nc.vector.tensor_mul(pair[0:64, :],
                                                 O[h01][0:64, :], rlb[:])
                        else:
                            ot = stg.tile([64, 512], BF16, tag="ot", bufs=2)
                            nc.vector.tensor_mul(ot[:], O[h01][0:64, :],
                                                 rlb[:])
                            nc.sync.dma_start(pair[64:128, :], ot[:])
                    otn_tiles[hp] = pair
                return copy_part, norm

            for hp in range(4):
                flush((qg, hp, 0))
                O = [ps_o.tile([65, 512], F32, tag=f"O{h01}", name="O")
                     for h01 in range(2)]
                pend = None

                def emit_av(ent, O=O, hp=hp):
                    c, pt, qoff, N = ent
                    for h01 in range(2):
                        h = 2 * hp + h01
                        nc.tensor.matmul(
                            O[h01][:, qoff:qoff + N],
                            vsb[c][:, 65 * h: 65 * h + 65],
                            pt[:, 512 * h01: 512 * h01 + N],
                            start=(c == 0), stop=(c == nchunks - 1))

                for c in range(nchunks):
                    if c > 0:
                        flush((qg, hp, c))
                    cmod = c - 4 * qg
                    qoff = 128 * cmod if cmod >= 0 else 0
                    N = 512 - qoff
                    sp = ps_s.tile([128, 1024], F32, tag="S")
                    for h01 in range(2):
                        base = 64 * h01
                        nc.tensor.matmul(
                            sp[:, 512 * h01: 512 * h01 + N],
                            krh[hp][base:base + 64, c * 128:(c + 1) * 128],
                            qrh[hp][base:base + 64, q0 + qoff:q0 + qoff + N],
                            start=True, stop=True,
                            tile_position=(base, 0))
                    if cmod >= 0 and mask_mm:
                        # causal mask: accumulate -3e4 onto the upper
                        # triangle of both heads' diagonal 128-col regions
                        if mask_merge:
                            spm = sp[:].rearrange("p (two n) -> p two n",
                                                  two=2)
                            nc.tensor.matmul(
                                spm[:, :, 0:128], trid, eye2,
                                start=False, stop=True)
                        else:
                            for h01 in range(2):
                                nc.tensor.matmul(
                                    sp[:, 512 * h01: 512 * h01 + 128],
                                    trid, eye2[:, 0:128],
                                    start=False, stop=True)
                    pt = ppool.tile([128, 1024], BF16, tag="pt", bufs=8)
                    if N == 512:
                        nc.scalar.activation(
                            pt[:], sp[:],
                            mybir.ActivationFunctionType.Exp, scale=0.125,
                            bias=zbias[:])
                    else:
                        sp3 = sp[:].rearrange("p (two n) -> p two n", two=2)
                        pt3 = pt[:].rearrange("p (two n) -> p two n", two=2)
                        nc.scalar.activation(
                            pt3[:, :, 0:N], sp3[:, :, 0:N],
                            mybir.ActivationFunctionType.Exp, scale=0.125,
                            bias=zbias[:])
                    if cmod >= 0 and not mask_mm:
                        # causal mask: zero upper triangle post-exp
                        pt3 = pt[:].rearrange("p (two n) -> p two n", two=2)
                        nc.gpsimd.tensor_mul(pt3[:, :, 0:128],
                                             pt3[:, :, 0:128], m3[:])
                    # deferred work between scores/exp and attn@v: the
                    # previous hp's normalize MUST be emitted before this
                    # hp's first attn@v (WAR on the O ring); projection
                    # fill units spread across later chunks.
                    if c == 1 and prev_norm is not None:
                        prev_norm()
                        prev_norm = None
                    else:
                        rhythm[0] += 1
                        if rhythm[0] % rhythm_n == 0 and pending:
                            pending.pop(0)[1]()
                    if pend is not None:
                        emit_av(pend)
                    pend = (c, pt, qoff, N)
                emit_av(pend)

                if prev_norm is not None:
                    prev_norm()
                copy_part, prev_norm = make_norm(hp, O)
                if norm_split:
                    copy_part()
                else:
                    prev_norm = _chain(copy_part, prev_norm)
                if pending:
                    pending.pop(0)[1]()

            prev_norm()
            return otn_tiles

        def oproj_thunk(qg, otn_tiles, qt):
            def run():
                qtile = qg * 4 + qt
                osb = stg.tile([128, 1024], BF16, tag="osb", bufs=3)
                for oh in range(2):
                    f = ps_x.tile([128, 512], F32, tag="scr", name="F")
                    for p in range(4):
                        nc.tensor.matmul(
                            f[:],
                            otn_tiles[p][:, qt * 128:(qt + 1) * 128],
                            woh[:, p * D + oh * 512:
                                p * D + oh * 512 + 512],
                            start=(p == 0), stop=(p == 3))
                    nc.vector.tensor_copy(
                        osb[:, oh * 512:(oh + 1) * 512], f[:])
                nc.sync.dma_start(
                    out[qtile * 128:(qtile + 1) * 128, :], osb[:])
            return run

        for _rep in range(reps):
            xts = {}

            def load_xt(sc, halves=False):
                xt = xpool.tile([128, 8 * 512], BF16, tag="xt")
                if halves:
                    nc.sync.dma_start(xt[:, 0:2048],
                                      x8d[:, sc * 4096:sc * 4096 + 2048])
                    nc.sync.dma_start(xt[:, 2048:4096],
                                      x8d[:, sc * 4096 + 2048:
                                          (sc + 1) * 4096])
                else:
                    nc.sync.dma_start(xt[:],
                                      x8d[:, sc * 4096:(sc + 1) * 4096])
                xts[sc] = xt[:].rearrange("p (k s) -> p k s", k=8)

            def qm(sc, m, fast=False):
                def run():
                    qs = proj_chunk(wq3, xts[sc], m, "q", raw=fast)
                    rope_chunk(qs, qrh, sc * 512, m, fast=fast)
                return run

            def km(sc, m, fast=False):
                def run():
                    ks = proj_chunk(wk3, xts[sc], m, "k", raw=fast)
                    rope_chunk(ks, krh, sc * 512, m, fast=fast)
                return run

            def vu(sc, st):
                return lambda: vproj_chunk(xts[sc], sc, st)

            def block_entries(sc, skip=0):
                """(deadline, thunk) units for block sc, FIFO in deadline
                order.  qm/km of chunk m due before attention(sc) hp m;
                v s-chunk st due before attn@v of kv chunk 4sc+st."""
                ents = [((sc, 0, 0), qm(sc, 0)), ((sc, 0, 0), km(sc, 0))]
                for st in range(4):
                    dl = min(4 * sc + st + 1, 4 * sc + 3)
                    ents.append(((sc, 0, dl), vu(sc, st)))
                for m in range(1, 4):
                    ents.append(((sc, m, 0), qm(sc, m)))
                    ents.append(((sc, m, 0), km(sc, m)))
                return ents[skip:]

            load_xt(0)
            if _rep == 0:
                # startup-critical first: x block 0, then wq/wk chunk 0 +
                # block-0 cos|sin; bulk loads trail (SWDGE for weights)
                nc.sync.dma_start(wq[:, 0:1024], wq8[:, 0:1024])
                nc.scalar.dma_start(cst[:, 0:1024], cs8[:, 0:1024])
                nc.sync.dma_start(wk[:, 0:1024], wk8[:, 0:1024])
                nc.scalar.dma_start(maskt[:], maskd[:])
                nc.sync.dma_start(wv[:], wv8[:])
            load_xt(1)
            if _rep == 0:
                nc.scalar.dma_start(cst[:, 1024:4096], cs8[:, 1024:4096])
                nc.sync.dma_start(wq[:, 1024:4096], wq8[:, 1024:4096])
                nc.sync.dma_start(wk[:, 1024:4096], wk8[:, 1024:4096])
                nc.scalar.dma_start(woh[:], wod[:])

            # slim prologue: just q/k chunk 0 of block 0 (latency-fast
            # rope); everything else flows through the deadline FIFO
            qm(0, 0, fast=True)()
            km(0, 0, fast=True)()
            pending = block_entries(0, skip=2)
            pending += block_entries(1)

            for qg in range(4):
                otn_tiles = attention(qg, pending)
                # stagger o_proj deadlines (FIFO stays deadline-ordered)
                odls = [(qg + 1, 3, 1), (qg + 1, 3, 2), (qg + 1, 3, 3),
                        (qg + 2, 0, 0)]
                pending.extend((odls[qt], oproj_thunk(qg, otn_tiles, qt))
                               for qt in range(4))
                if qg == 0:
                    load_xt(2)
                    pending += block_entries(2)
                elif qg == 1:
                    load_xt(3)
                    pending += block_entries(3)
            for _dl, thunk in pending:
                thunk()

    nc.compile()
    return nc


def _prep_in_maps(x, token_positions, Wq, Wk, Wv, Wo):
    BF = mybir.dt.np(BF16)
    half = DK // 2
    freqs = (1.0 / (THETA ** (2.0 * np.arange(half, dtype=np.float32) / DK)))
    angles = token_positions.astype(np.float32)[:, None] * freqs[None, :]
    cos = np.cos(angles).astype(np.float32).T    # [32, S]
    sin = np.sin(angles).astype(np.float32).T
    # natural dk order: rows (2j, 2j+1) both carry freq j; sin sign-folded
    # (+ on even rows, - on odd) so post-shuffle r = x*cos + shuf(x*sin)
    cos2 = np.repeat(cos, 2, axis=0)             # [64, S]
    sin2 = np.repeat(sin, 2, axis=0).copy()
    sin2[1::2, :] *= -1.0
    cos4 = np.tile(cos2, (2, 1))                 # [128, S]
    sin4 = np.tile(sin2, (2, 1))
    # interleave per 512-block: [128, blk(4) x (cos 512 | sin 512)]
    cs8 = np.ascontiguousarray(
        np.stack([cos4.reshape(128, 4, 512), sin4.reshape(128, 4, 512)],
                 axis=2).reshape(128, 2 * S)).astype(BF)

    kv_l = np.arange(128)[:, None]
    q_l = np.arange(128)[None, :]
    trid = np.where(q_l > kv_l, -30000.0, 0.0).astype(np.float32)
    eye = np.eye(128, dtype=np.float32)
    m1 = (q_l >= kv_l).astype(np.float32)
    maskd = np.concatenate([trid, eye, eye, m1, m1], axis=1).astype(BF)

    def chunked(wT, nk):
        # [nk*128, M] -> [128, nk*M] kk-chunk-major
        m = wT.shape[1]
        return np.ascontiguousarray(
            wT.reshape(nk, 128, m).transpose(1, 0, 2).reshape(128, nk * m))

    def chunked_m(wT):
        # [1024, 512] -> [128, m(4) x t(8) x 128] m-chunk-major
        return np.ascontiguousarray(
            wT.reshape(8, 128, 4, 128).transpose(1, 2, 0, 3)
            .reshape(128, 4096))

    in_maps = []
    for c in range(8):
        b, g = c // 2, c % 2
        rows = slice(g * QR, (g + 1) * QR)
        wq_g = Wq[rows, :].T             # [D, QR], natural head-major rows
        wk_g = Wk[rows, :].T
        wv_g = Wv[rows, :].T
        xT = x[b].T                      # [D, S]
        x8 = np.ascontiguousarray(
            xT.reshape(8, 128, 4, 512).transpose(1, 2, 0, 3)
            .reshape(128, 4 * 8 * 512)).astype(BF)
        woT = Wo[:, rows].T              # [QR, D]
        wod = np.ascontiguousarray(
            woT.reshape(4, 128, D).transpose(1, 0, 2).reshape(128, 4 * D)
        ).astype(BF)
        blob = np.concatenate(
            [x8, chunked_m(wq_g).astype(BF), chunked_m(wk_g).astype(BF),
             chunked(wv_g, 8).astype(BF), wod, cs8, maskd], axis=1)
        in_maps.append({"blob": np.ascontiguousarray(blob)})
    return in_maps


def kernel(x, token_positions, Wq, Wk, Wv, Wo):
    global _COMPILED
    x = np.asarray(x, dtype=np.float32)
    token_positions = np.asarray(token_positions)
    Wq = np.asarray(Wq, dtype=np.float32)
    Wk = np.asarray(Wk, dtype=np.float32)
    Wv = np.asarray(Wv, dtype=np.float32)
    Wo = np.asarray(Wo, dtype=np.float32)

    if _COMPILED is None:
        _COMPILED = build_kernel()
    nc = _COMPILED

    in_maps = _prep_in_maps(x, token_positions, Wq, Wk, Wv, Wo)
    res = run_bass_kernel_spmd(nc, in_maps, core_ids=list(range(8)))

    out = np.empty((B, S, D), dtype=np.float32)
    for b in range(B):
        out[b] = (res.results[2 * b]["out"].astype(np.float32)
                  + res.results[2 * b + 1]["out"].astype(np.float32))
    return out


def time_device(inputs, n1=32, n2=128, repeats=2):
    """Async-pipelined device timing: enqueue N executions of the sharded
    PJRT call with device-resident inputs, block once.  The marginal
    (T(n2)-T(n1))/(n2-n1) cancels per-dispatch axon overhead and
    approximates per-execution hardware time.  Returns ns."""
    import time

    import jax
    from jax.sharding import Mesh, NamedSharding, PartitionSpec

    try:
        from jax.experimental.shard_map import shard_map
    except ImportError:
        from jax import shard_map

    from concourse import bass2jax

    global _COMPILED
    if _COMPILED is None:
        _COMPILED = build_kernel()
    nc = _COMPILED
    bass2jax.install_neuronx_cc_hook()

    in_maps = _prep_in_maps(
        np.asarray(inputs["x"], np.float32), np.asarray(inputs["token_positions"]),
        np.asarray(inputs["Wq"], np.float32), np.asarray(inputs["Wk"], np.float32),
        np.asarray(inputs["Wv"], np.float32), np.asarray(inputs["Wo"], np.float32))

    partition_name = (nc.partition_id_tensor.name
                      if nc.partition_id_tensor else None)
    in_names, out_names, out_avals, zero_outs = [], [], [], []
    for alloc in nc.m.functions[0].allocations:
        if not isinstance(alloc, mybir.MemoryLocationSet):
            continue
        name = alloc.memorylocations[0].name
        if alloc.kind == "ExternalInput":
            if name != partition_name:
                in_names.append(name)
        elif alloc.kind == "ExternalOutput":
            out_names.append(name)
            shape = tuple(alloc.tensor_shape)
            dtype = mybir.dt.np(alloc.dtype)
            out_avals.append(jax.core.ShapedArray(shape, dtype))
            zero_outs.append(np.zeros(shape, dtype))
    n_params = len(in_names)
    all_in_names = in_names + out_names
    if partition_name is not None:
        all_in_names = all_in_names + [partition_name]

    def _body(*args):
        operands = list(args)
        if partition_name is not None:
            operands.append(bass2jax.partition_id_tensor())
        outs = bass2jax._bass_exec_p.bind(
            *operands,
            out_avals=tuple(out_avals),
            in_names=tuple(all_in_names),
            out_names=tuple(out_names),
            lowering_input_output_aliases=(),
            sim_require_finite=True,
            sim_require_nnan=True,
            nc=nc,
        )
        return tuple(outs)

    n_cores = 8
    devices = jax.devices()[:n_cores]
    mesh = Mesh(np.asarray(devices), ("core",))
    spec = PartitionSpec("core")
    sharded = jax.jit(
        shard_map(_body, mesh=mesh,
                  in_specs=(spec,) * (n_params + len(out_names)),
                  out_specs=(spec,) * len(out_names), check_rep=False))
    sharding = NamedSharding(mesh, spec)
    dev_args = [
        jax.device_put(
            np.concatenate([np.asarray(in_maps[c][nm]) for c in range(n_cores)],
                           axis=0), sharding)
        for nm in in_names
    ] + [
        jax.device_put(
            np.zeros((n_cores * z.shape[0], *z.shape[1:]), z.dtype), sharding)
        for z in zero_outs
    ]

    jax.block_until_ready(sharded(*dev_args))

    def run_batch(n):
        t0 = time.perf_counter()
        outs = None
        for _ in range(n):
            outs = sharded(*dev_args)
        jax.block_until_ready(outs)
        return time.perf_counter() - t0

    best = None
    for _ in range(repeats):
        ta = run_batch(n1)
        tb = run_batch(n2)
        marg = (tb - ta) / (n2 - n1)
        best = marg if best is None else min(best, marg)
    return best * 1e9



# revision 71
# speedup vs baseline: 1.2871x; 1.0255x over previous
"""Trainium2 Bass kernel for causal multi-head attention with RoPE.

Problem: B=4, S=2048, D=1024, H=16, DK=64 dense transformer attention
(q/k/v projections -> interleaved RoPE on q,k -> causal softmax attention
-> output projection), fp32 inputs/outputs.

Sharding: 8 NeuronCores, core c handles batch b=c//2 and head-group
g=c%2 (8 of the 16 heads).  Each core computes a partial o_proj output
for its batch over its heads; the host sums the two partials per batch.

Kernel design (per core) — bf16 data path, Act-engine-bound pipeline:
  - All matmul operands bf16 (HW-measured ~0.62 cyc/row, slightly faster
    than f32r; fp8-DoubleRow measured 2.3x SLOWER on HW, so not used).
    Host delivers x/W in bf16 kk-chunk-major layouts; PSUM stays fp32.
  - RoPE in bf16 directly in head-contiguous layout: W rows are kept in
    natural (head, dk) order, so the interleaved (2j, 2j+1) pair swap is
    a stream_shuffle (XOR-1 mask within 32-partition blocks) on DVE plus
    two muls and an add against repeated-j cos / sign-folded sin tables.
    No DMA permutes; rope output writes the persistent qrh/krh tiles.
  - scores: S_ps[kv, q] = k_chunk @ qT, two heads per PE pass via
    tile_position row groups (K=64 each), both heads' scores in one
    [128, 1024] PSUM tile (2 banks); ONE merged exp per (hp, chunk)
    -> pt bf16 (the ~400ns fixed Act-instruction overhead dominates,
    so fewer/bigger exps win; the exp stream is the kernel's
    bottleneck engine at ~1.04 ns/col).
  - causal mask applied PRE-exp as an extra PE matmul accumulating
    -3e4 * upper_tri into the diagonal 128-col region of the scores
    PSUM (stationary = -3e4*tri, moving = I128); exp underflows the
    masked region to 0, keeping Pool/DVE out of the Act->PE chain.
  - v stored bf16 with a per-head ones-column (65 cols/head) so attn@v
    also produces the softmax denominator row; attn@v in bf16.
  - software pipelining: scores/exp of chunk c+1 issue before attn@v of
    chunk c, so the in-order PE queue never parks waiting on the exp.
  - deadline-FIFO fill scheduling: all projection / o_proj work units
    live in one FIFO of (deadline, thunk); each unit is emitted either
    when its latest-safe position (qg, hp, chunk) arrives (correctness:
    q/k chunk m before attention hp m, v s-chunks before the diagonal
    attn@v, o_proj before the otn ring wraps) or by a rhythm pop every
    3rd chunk slot, pacing ~1.7us proj units against ~1.5us exps so
    neither PE nor Act starves.  HW A/B showed every-2nd-chunk pops
    starve Act (PE:Act busy ratio is ~1.66).
  - slim prologue: only q/k chunk 0 of block 0 are emitted directly
    (latency-fast variant: rope reads the proj PSUM directly and stays
    entirely on DVE); first exp lands ~15us into the kernel instead of
    ~47us.  Startup DMAs are ordered x block 0 > wq/wk chunk 0 + cos/sin
    block 0 > wv > everything else; exp bias is an explicit memset tile
    (a float bias lowers to a const-AP DMA on the Act queue that made
    every exp transitively wait on table loads).
  - normalize: copy the two l rows into one SBUF tile at the hp's end
    (the DVE drains them during the next hp's chunk 0), then DEFERRED at
    chunk 1 of the next hp (before its first attn@v -- WAR on the O
    ring): K=1 ones-matmul broadcast, reciprocal + multiply on DVE ->
    normalized bf16 pair tile per hp.
  - o_proj in bf16 accumulated in PSUM; outputs DMA'd as bf16, host sums
    the two per-batch partials in fp32.  o_proj(qg) units carry
    staggered deadlines across attention(qg+1)'s last hp so they never
    flush as one 9us PE burst.
  - PSUM budget (8 banks): shared scratch ring (proj/o_proj/rbp) 2 +
    scores 2x2 + O accumulators 2.
  - single input blob [128, 37504] (x | wq | wk | wv | wo | cos/sin |
    mask), sliced by AP in-kernel: fewer per-dispatch PJRT buffers.
"""

import sys

sys.path.insert(0, "/opt/trn_rl_repo")

from contextlib import ExitStack

import numpy as np

import concourse.bass as bass
import concourse.tile as tile
from concourse import bacc, mybir
from concourse.bass_utils import run_bass_kernel_spmd

B, S, D, H = 4, 2048, 1024, 16
DK = D // H          # 64
NHL = 8              # heads per core (local)
QR = NHL * DK        # 512 projected rows per core
NKC = S // 128       # 16 kv chunks
THETA = 10000.0

F32 = mybir.dt.float32
BF16 = mybir.dt.bfloat16

_COMPILED = None

BENCH_VARIANTS = [("r6", {"reps": 6}),
                  ("r6_pull", {"reps": 6, "dl_pull": True}),
                  ("r6_allfast", {"reps": 6, "allfast": True})]

# stream_shuffle mask: swap partition pairs (2j, 2j+1) within each 32-block
_SWAP_MASK = [i ^ 1 for i in range(32)]


def _chain(f, g):
    def run():
        f()
        g()
    return run


def build_kernel(reps=1, mask_mm=True, mask_merge=True, rope_t1_pool=True,
                 norm_split=True, rhythm_n=3, dl_pull=False, allfast=False):
    nc = bacc.Bacc("TRN2", target_bir_lowering=False, debug=False,
                   enable_asserts=False)

    # single input blob [128, 37504]: x8 | wq8 | wk8 | wv8 | wod | cs8 |
    # maskd concatenated along the free dim (fewer per-dispatch buffers).
    # wq8/wk8 are m-chunk-major: [128, m(4) x t(8) x 128] so per-m slices
    # are contiguous 2KB/partition loads; wv8 stays t-major (consumed as
    # full-width moving operands); cs8 = cos|sin interleaved per 512-block
    blob = nc.dram_tensor("blob", [128, 37504], BF16,
                          kind="ExternalInput").ap()
    _off = [0]

    def _slc(n):
        a = _off[0]
        _off[0] += n
        return blob[:, a:a + n]

    x8d = _slc(4 * 8 * 512)
    wq8 = _slc(8 * QR)
    wk8 = _slc(8 * QR)
    wv8 = _slc(8 * QR)
    wod = _slc(4 * D)
    cs8 = _slc(2 * S)
    maskd = _slc(640)
    out = nc.dram_tensor("out", [S, D], BF16, kind="ExternalOutput").ap()

    with tile.TileContext(nc) as tc, ExitStack() as ctx:
        persist = ctx.enter_context(tc.tile_pool(name="persist", bufs=1))
        # head-contiguous rope'd q/k: chunk hp holds heads (2hp, 2hp+1);
        # within a head: [even-lane j 0..31 ; odd-lane j 0..31]
        qrh = [persist.tile([128, S], BF16, tag=f"qrh{i}", name=f"qrh{i}")
               for i in range(4)]
        krh = [persist.tile([128, S], BF16, tag=f"krh{i}", name=f"krh{i}")
               for i in range(4)]
        # v natural layout, 65 cols per head (64 v + ones), all 16 s-tiles
        v_all = persist.tile([128, NKC * NHL * 65], BF16, tag="v_all")
        vsb = [v_all[:, i * NHL * 65:(i + 1) * NHL * 65] for i in range(NKC)]
        maskt = persist.tile([128, 640], BF16, tag="maskt")
        onest = persist.tile([65, 64], BF16, tag="onest")
        cst = persist.tile([128, 2 * S], BF16, tag="cst")
        wq = persist.tile([128, 8 * QR], BF16, tag="wq")
        wk = persist.tile([128, 8 * QR], BF16, tag="wk")
        wv = persist.tile([128, 8 * QR], BF16, tag="wv")
        woh = persist.tile([128, 4 * D], BF16, tag="woh")

        trid = maskt[:, 0:128]
        eye2 = maskt[:, 128:384]
        m3 = maskt[:, 384:640].rearrange("p (two n) -> p two n", two=2)
        nc.vector.memset(onest[:], 1.0)
        # explicit zero bias for exp: a float bias would lower to a const
        # AP DMA'd on the Act queue, making every exp wait on that queue's
        # preceding table loads
        zbias = persist.tile([128, 1], F32, tag="zbias")
        nc.vector.memset(zbias[:], 0.0)
        # ones column (col 64 of each head's 65-col block), all kv tiles
        v3 = v_all[:].rearrange("p (n c) -> p n c", c=65)
        nc.gpsimd.memset(v3[:, :, 64:65], 1.0)

        wq3 = wq[:].rearrange("p (m k c) -> p m k c", m=4, k=8)
        wk3 = wk[:].rearrange("p (m k c) -> p m k c", m=4, k=8)
        wv3 = wv[:].rearrange("p (k q) -> p k q", k=8)

        xpool = ctx.enter_context(tc.tile_pool(name="xp", bufs=3))
        stg = ctx.enter_context(tc.tile_pool(name="stg", bufs=2))
        ppool = ctx.enter_context(tc.tile_pool(name="pt", bufs=6))
        otn = ctx.enter_context(tc.tile_pool(name="otn", bufs=8))
        # PSUM budget (8 banks): scratch 2 + sp 2x2 + O 2
        ps_x = ctx.enter_context(
            tc.tile_pool(name="ps_x", bufs=2, space="PSUM"))
        ps_s = ctx.enter_context(
            tc.tile_pool(name="ps_s", bufs=2, space="PSUM"))
        ps_o = ctx.enter_context(
            tc.tile_pool(name="ps_o", bufs=1, space="PSUM"))

        def proj_chunk(w3, xt3, m, names, raw=False):
            """One m-chunk [128, 512] of a q/k projection, bf16 (raw=True
            returns the f32 PSUM tile without staging to SBUF)."""
            ps = ps_x.tile([128, 512], F32, tag="scr", name="pps")
            for t in range(8):
                nc.tensor.matmul(
                    ps[:],
                    w3[:, m, t, :],
                    xt3[:, t, :],
                    start=(t == 0), stop=(t == 7))
            if raw:
                return ps
            qs = stg.tile([128, 512], BF16, tag="qps", bufs=12,
                          name=f"{names}{m}")
            nc.vector.tensor_copy(qs[:], ps[:])
            return qs

        def vproj_chunk(xt3, sc, st):
            vp = ps_x.tile([128, 512], F32, tag="scr", name="vp")
            for t in range(8):
                nc.tensor.matmul(
                    vp[:],
                    xt3[:, t, st * 128:(st + 1) * 128],
                    wv3[:, t, :],
                    start=(t == 0), stop=(t == 7))
            vdst = vsb[sc * 4 + st][:].rearrange(
                "p (h c) -> p h c", c=65)[:, :, 0:64]
            vsrc = vp[:].rearrange("p (h c) -> p h c", c=64)
            nc.vector.tensor_copy(vdst, vsrc)

        def rope_chunk(sbm, dst, s0, m, fast=False):
            """RoPE one proj m-chunk [128, 512] (heads 2m, 2m+1 in natural
            dk order) straight into dst[m][:, s0:s0+512]: pair-swap via
            stream_shuffle, sign folded into the sin table.  fast=True
            keeps the whole chain on DVE (no cross-engine hops) for
            latency-critical prologue chunks."""
            costc = cst[:, 2 * s0:2 * s0 + 512]
            sintc = cst[:, 2 * s0 + 512:2 * s0 + 1024]
            t2 = stg.tile([128, 512], BF16, tag="tmp", bufs=6)
            pool_mul = (nc.vector if fast else nc.gpsimd).tensor_mul
            pool_mul(t2[:], sbm[:], sintc)
            ss = stg.tile([128, 512], BF16, tag="tmp", bufs=6)
            nc.vector.stream_shuffle(ss[:], t2[:], _SWAP_MASK)
            t1 = stg.tile([128, 512], BF16, tag="tmp", bufs=6)
            if rope_t1_pool and not fast:
                nc.gpsimd.tensor_mul(t1[:], sbm[:], costc)
            else:
                nc.vector.tensor_mul(t1[:], sbm[:], costc)
            nc.vector.tensor_add(dst[m][:, s0:s0 + 512], t1[:], ss[:])

        rhythm = [0]

        def attention(qg, pending):
            """One q-group's attention.  `pending` is a FIFO of
            (deadline, thunk) units: deadline (qg, hp, c) = latest global
            position the unit must be emitted at (correctness); a rhythm
            pop additionally drains one unit every ~3rd chunk so proj PE
            work is paced evenly against the exp stream."""
            q0 = qg * 512
            nchunks = 4 * qg + 4
            otn_tiles = [None] * 4
            prev_norm = None      # deferred normalize of the previous hp

            def flush(pos):
                while pending and pending[0][0] <= pos:
                    pending.pop(0)[1]()

            def make_norm(hp, O):
                """Returns (copy_part, norm): copy_part runs at the hp's
                end (so the DVE drains the l-row copies during the next
                hp's chunk 0), norm is deferred to chunk 1 of the next hp
                (before its first attn@v -- WAR on the O ring)."""
                state = {}

                def copy_part():
                    lsb = stg.tile([65, 1024], BF16, tag="lsb", bufs=2)
                    nc.vector.tensor_copy(lsb[64:65, 0:512],
                                          O[0][64:65, :])
                    nc.vector.tensor_copy(lsb[64:65, 512:1024],
                                          O[1][64:65, :])
                    state["lsb"] = lsb

                def norm():
                    lsb = state["lsb"]
                    pair = otn.tile([128, 512], BF16, tag="pair", bufs=8,
                                    name="pair")
                    for h01 in range(2):
                        rbp = ps_x.tile([128, 512], F32, tag="scr",
                                        name="rbp")
                        nc.tensor.matmul(rbp[0:64, :],
                                         onest[64:65, 0:64],
                                         lsb[64:65,
                                             512 * h01:512 * h01 + 512],
                                         start=True, stop=True)
                        rlb = stg.tile([64, 512], F32, tag="rlb", bufs=2)
                        nc.vector.reciprocal(rlb[:], rbp[0:64, :])
                        if h01 == 0:
                            nc.vector.tensor_mul(pair[0:64, :],
                                                 O[h01][0:64, :], rlb[:])
                        else:
                            ot = stg.tile([64, 512], BF16, tag="ot", bufs=2)
                            nc.vector.tensor_mul(ot[:], O[h01][0:64, :],
                                                 rlb[:])
                            nc.sync.dma_start(pair[64:128, :], ot[:])
                    otn_tiles[hp] = pair
                return copy_part, norm

            for hp in range(4):
                flush((qg, hp, 0))
                O = [ps_o.tile([65, 512], F32, tag=f"O{h01}", name="O")
                     for h01 in range(2)]
                pend = None

                def emit_av(ent, O=O, hp=hp):
                    c, pt, qoff, N = ent
                    for h01 in range(2):
                        h = 2 * hp + h01
                        nc.tensor.matmul(
                            O[h01][:, qoff:qoff + N],
                            vsb[c][:, 65 * h: 65 * h + 65],
                            pt[:, 512 * h01: 512 * h01 + N],
                            start=(c == 0), stop=(c == nchunks - 1))

                for c in range(nchunks):
                    if c > 0:
                        flush((qg, hp, c))
                    cmod = c - 4 * qg
                    qoff = 128 * cmod if cmod >= 0 else 0
                    N = 512 - qoff
                    sp = ps_s.tile([128, 1024], F32, tag="S")
                    for h01 in range(2):
                        base = 64 * h01
                        nc.tensor.matmul(
                            sp[:, 512 * h01: 512 * h01 + N],
                            krh[hp][base:base + 64, c * 128:(c + 1) * 128],
                            qrh[hp][base:base + 64, q0 + qoff:q0 + qoff + N],
                            start=True, stop=True,
                            tile_position=(base, 0))
                    if cmod >= 0 and mask_mm:
                        # causal mask: accumulate -3e4 onto the upper
                        # triangle of both heads' diagonal 128-col regions
                        if mask_merge:
                            spm = sp[:].rearrange("p (two n) -> p two n",
                                                  two=2)
                            nc.tensor.matmul(
                                spm[:, :, 0:128], trid, eye2,
                                start=False, stop=True)
                        else:
                            for h01 in range(2):
                                nc.tensor.matmul(
                                    sp[:, 512 * h01: 512 * h01 + 128],
                                    trid, eye2[:, 0:128],
                                    start=False, stop=True)
                    pt = ppool.tile([128, 1024], BF16, tag="pt", bufs=8)
                    if N == 512:
                        nc.scalar.activation(
                            pt[:], sp[:],
                            mybir.ActivationFunctionType.Exp, scale=0.125,
                            bias=zbias[:])
                    else:
                        sp3 = sp[:].rearrange("p (two n) -> p two n", two=2)
                        pt3 = pt[:].rearrange("p (two n) -> p two n", two=2)
                        nc.scalar.activation(
                            pt3[:, :, 0:N], sp3[:, :, 0:N],
                            mybir.ActivationFunctionType.Exp, scale=0.125,
                            bias=zbias[:])
                    if cmod >= 0 and not mask_mm:
                        # causal mask: zero upper triangle post-exp
                        pt3 = pt[:].rearrange("p (two n) -> p two n", two=2)
                        nc.gpsimd.tensor_mul(pt3[:, :, 0:128],
                                             pt3[:, :, 0:128], m3[:])
                    # deferred work between scores/exp and attn@v: the
                    # previous hp's normalize MUST be emitted before this
                    # hp's first attn@v (WAR on the O ring); projection
                    # fill units spread across later chunks.
                    if c == 1 and prev_norm is not None:
                        prev_norm()
                        prev_norm = None
                    else:
                        rhythm[0] += 1
                        if rhythm[0] % rhythm_n == 0 and pending:
                            pending.pop(0)[1]()
                    if pend is not None:
                        emit_av(pend)
                    pend = (c, pt, qoff, N)
                emit_av(pend)

                if prev_norm is not None:
                    prev_norm()
                copy_part, prev_norm = make_norm(hp, O)
                if norm_split:
                    copy_part()
                else:
                    prev_norm = _chain(copy_part, prev_norm)
                if pending:
                    pending.pop(0)[1]()

            prev_norm()
            return otn_tiles

        def oproj_thunk(qg, otn_tiles, qt):
            def run():
                qtile = qg * 4 + qt
                osb = stg.tile([128, 1024], BF16, tag="osb", bufs=3)
                for oh in range(2):
                    f = ps_x.tile([128, 512], F32, tag="scr", name="F")
                    for p in range(4):
                        nc.tensor.matmul(
                            f[:],
                            otn_tiles[p][:, qt * 128:(qt + 1) * 128],
                            woh[:, p * D + oh * 512:
                                p * D + oh * 512 + 512],
                            start=(p == 0), stop=(p == 3))
                    nc.vector.tensor_copy(
                        osb[:, oh * 512:(oh + 1) * 512], f[:])
                nc.sync.dma_start(
                    out[qtile * 128:(qtile + 1) * 128, :], osb[:])
            return run

        for _rep in range(reps):
            xts = {}

            def load_xt(sc, halves=False):
                xt = xpool.tile([128, 8 * 512], BF16, tag="xt")
                if halves:
                    nc.sync.dma_start(xt[:, 0:2048],
                                      x8d[:, sc * 4096:sc * 4096 + 2048])
                    nc.sync.dma_start(xt[:, 2048:4096],
                                      x8d[:, sc * 4096 + 2048:
                                          (sc + 1) * 4096])
                else:
                    nc.sync.dma_start(xt[:],
                                      x8d[:, sc * 4096:(sc + 1) * 4096])
                xts[sc] = xt[:].rearrange("p (k s) -> p k s", k=8)

            def qm(sc, m, fast=False):
                f = fast or allfast

                def run():
                    qs = proj_chunk(wq3, xts[sc], m, "q", raw=f)
                    rope_chunk(qs, qrh, sc * 512, m, fast=f)
                return run

            def km(sc, m, fast=False):
                f = fast or allfast

                def run():
                    ks = proj_chunk(wk3, xts[sc], m, "k", raw=f)
                    rope_chunk(ks, krh, sc * 512, m, fast=f)
                return run

            def vu(sc, st):
                return lambda: vproj_chunk(xts[sc], sc, st)

            def block_entries(sc, skip=0):
                """(deadline, thunk) units for block sc, FIFO in deadline
                order.  qm/km of chunk m due before attention(sc) hp m;
                v s-chunk st due before attn@v of kv chunk 4sc+st.  With
                dl_pull, hp m's q/k units are due 2 chunks before the hp
                boundary so their rope chains overlap the previous hp's
                tail instead of stalling the next scores."""
                last = 4 * sc + 3
                if dl_pull:
                    ents = [((sc, 0, 0), qm(sc, 0)), ((sc, 0, 0), km(sc, 0)),
                            ((sc, 0, min(4 * sc + 1, last)), vu(sc, 0)),
                            ((sc, 0, last - 2), qm(sc, 1)),
                            ((sc, 0, min(4 * sc + 2, last)), vu(sc, 1)),
                            ((sc, 0, last - 1), km(sc, 1)),
                            ((sc, 0, last), vu(sc, 2)),
                            ((sc, 0, last), vu(sc, 3))]
                    for m in range(2, 4):
                        ents.append(((sc, m - 1, last - 2), qm(sc, m)))
                        ents.append(((sc, m - 1, last - 1), km(sc, m)))
                    return ents[skip:]
                ents = [((sc, 0, 0), qm(sc, 0)), ((sc, 0, 0), km(sc, 0))]
                for st in range(4):
                    dl = min(4 * sc + st + 1, last)
                    ents.append(((sc, 0, dl), vu(sc, st)))
                for m in range(1, 4):
                    ents.append(((sc, m, 0), qm(sc, m)))
                    ents.append(((sc, m, 0), km(sc, m)))
                return ents[skip:]

            load_xt(0)
            if _rep == 0:
                # startup-critical first: x block 0, then wq/wk chunk 0 +
                # block-0 cos|sin; bulk loads trail (SWDGE for weights)
                nc.sync.dma_start(wq[:, 0:1024], wq8[:, 0:1024])
                nc.scalar.dma_start(cst[:, 0:1024], cs8[:, 0:1024])
                nc.sync.dma_start(wk[:, 0:1024], wk8[:, 0:1024])
                nc.scalar.dma_start(maskt[:], maskd[:])
                nc.sync.dma_start(wv[:], wv8[:])
            load_xt(1)
            if _rep == 0:
                nc.scalar.dma_start(cst[:, 1024:4096], cs8[:, 1024:4096])
                nc.sync.dma_start(wq[:, 1024:4096], wq8[:, 1024:4096])
                nc.sync.dma_start(wk[:, 1024:4096], wk8[:, 1024:4096])
                nc.scalar.dma_start(woh[:], wod[:])

            # slim prologue: just q/k chunk 0 of block 0 (latency-fast
            # rope); everything else flows through the deadline FIFO
            qm(0, 0, fast=True)()
            km(0, 0, fast=True)()
            pending = block_entries(0, skip=2)
            pending += block_entries(1)

            for qg in range(4):
                otn_tiles = attention(qg, pending)
                # stagger o_proj deadlines (FIFO stays deadline-ordered)
                odls = [(qg + 1, 3, 1), (qg + 1, 3, 2), (qg + 1, 3, 3),
                        (qg + 2, 0, 0)]
                pending.extend((odls[qt], oproj_thunk(qg, otn_tiles, qt))
                               for qt in range(4))
                if qg == 0:
                    load_xt(2)
                    pending += block_entries(2)
                elif qg == 1:
                    load_xt(3)
                    pending += block_entries(3)
            for _dl, thunk in pending:
                thunk()

    nc.compile()
    return nc


def _prep_in_maps(x, token_positions, Wq, Wk, Wv, Wo):
    BF = mybir.dt.np(BF16)
    half = DK // 2
    freqs = (1.0 / (THETA ** (2.0 * np.arange(half, dtype=np.float32) / DK)))
    angles = token_positions.astype(np.float32)[:, None] * freqs[None, :]
    cos = np.cos(angles).astype(np.float32).T    # [32, S]
    sin = np.sin(angles).astype(np.float32).T
    # natural dk order: rows (2j, 2j+1) both carry freq j; sin sign-folded
    # (+ on even rows, - on odd) so post-shuffle r = x*cos + shuf(x*sin)
    cos2 = np.repeat(cos, 2, axis=0)             # [64, S]
    sin2 = np.repeat(sin, 2, axis=0).copy()
    sin2[1::2, :] *= -1.0
    cos4 = np.tile(cos2, (2, 1))                 # [128, S]
    sin4 = np.tile(sin2, (2, 1))
    # interleave per 512-block: [128, blk(4) x (cos 512 | sin 512)]
    cs8 = np.ascontiguousarray(
        np.stack([cos4.reshape(128, 4, 512), sin4.reshape(128, 4, 512)],
                 axis=2).reshape(128, 2 * S)).astype(BF)

    kv_l = np.arange(128)[:, None]
    q_l = np.arange(128)[None, :]
    trid = np.where(q_l > kv_l, -30000.0, 0.0).astype(np.float32)
    eye = np.eye(128, dtype=np.float32)
    m1 = (q_l >= kv_l).astype(np.float32)
    maskd = np.concatenate([trid, eye, eye, m1, m1], axis=1).astype(BF)

    def chunked(wT, nk):
        # [nk*128, M] -> [128, nk*M] kk-chunk-major
        m = wT.shape[1]
        return np.ascontiguousarray(
            wT.reshape(nk, 128, m).transpose(1, 0, 2).reshape(128, nk * m))

    def chunked_m(wT):
        # [1024, 512] -> [128, m(4) x t(8) x 128] m-chunk-major
        return np.ascontiguousarray(
            wT.reshape(8, 128, 4, 128).transpose(1, 2, 0, 3)
            .reshape(128, 4096))

    in_maps = []
    for c in range(8):
        b, g = c // 2, c % 2
        rows = slice(g * QR, (g + 1) * QR)
        wq_g = Wq[rows, :].T             # [D, QR], natural head-major rows
        wk_g = Wk[rows, :].T
        wv_g = Wv[rows, :].T
        xT = x[b].T                      # [D, S]
        x8 = np.ascontiguousarray(
            xT.reshape(8, 128, 4, 512).transpose(1, 2, 0, 3)
            .reshape(128, 4 * 8 * 512)).astype(BF)
        woT = Wo[:, rows].T              # [QR, D]
        wod = np.ascontiguousarray(
            woT.reshape(4, 128, D).transpose(1, 0, 2).reshape(128, 4 * D)
        ).astype(BF)
        blob = np.concatenate(
            [x8, chunked_m(wq_g).astype(BF), chunked_m(wk_g).astype(BF),
             chunked(wv_g, 8).astype(BF), wod, cs8, maskd], axis=1)
        in_maps.append({"blob": np.ascontiguousarray(blob)})
    return in_maps


def kernel(x, token_positions, Wq, Wk, Wv, Wo):
    global _COMPILED
    x = np.asarray(x, dtype=np.float32)
    token_positions = np.asarray(token_positions)
    Wq = np.asarray(Wq, dtype=np.float32)
    Wk = np.asarray(Wk, dtype=np.float32)
    Wv = np.asarray(Wv, dtype=np.float32)
    Wo = np.asarray(Wo, dtype=np.float32)

    if _COMPILED is None:
        _COMPILED = build_kernel()
    nc = _COMPILED

    in_maps = _prep_in_maps(x, token_positions, Wq, Wk, Wv, Wo)
    res = run_bass_kernel_spmd(nc, in_maps, core_ids=list(range(8)))

    out = np.empty((B, S, D), dtype=np.float32)
    for b in range(B):
        out[b] = (res.results[2 * b]["out"].astype(np.float32)
                  + res.results[2 * b + 1]["out"].astype(np.float32))
    return out


def time_device(inputs, n1=32, n2=128, repeats=2):
    """Async-pipelined device timing: enqueue N executions of the sharded
    PJRT call with device-resident inputs, block once.  The marginal
    (T(n2)-T(n1))/(n2-n1) cancels per-dispatch axon overhead and
    approximates per-execution hardware time.  Returns ns."""
    import time

    import jax
    from jax.sharding import Mesh, NamedSharding, PartitionSpec

    try:
        from jax.experimental.shard_map import shard_map
    except ImportError:
        from jax import shard_map

    from concourse import bass2jax

    global _COMPILED
    if _COMPILED is None:
        _COMPILED = build_kernel()
    nc = _COMPILED
    bass2jax.install_neuronx_cc_hook()

    in_maps = _prep_in_maps(
        np.asarray(inputs["x"], np.float32), np.asarray(inputs["token_positions"]),
        np.asarray(inputs["Wq"], np.float32), np.asarray(inputs["Wk"], np.float32),
        np.asarray(inputs["Wv"], np.float32), np.asarray(inputs["Wo"], np.float32))

    partition_name = (nc.partition_id_tensor.name
                      if nc.partition_id_tensor else None)
    in_names, out_names, out_avals, zero_outs = [], [], [], []
    for alloc in nc.m.functions[0].allocations:
        if not isinstance(alloc, mybir.MemoryLocationSet):
            continue
        name = alloc.memorylocations[0].name
        if alloc.kind == "ExternalInput":
            if name != partition_name:
                in_names.append(name)
        elif alloc.kind == "ExternalOutput":
            out_names.append(name)
            shape = tuple(alloc.tensor_shape)
            dtype = mybir.dt.np(alloc.dtype)
            out_avals.append(jax.core.ShapedArray(shape, dtype))
            zero_outs.append(np.zeros(shape, dtype))
    n_params = len(in_names)
    all_in_names = in_names + out_names
    if partition_name is not None:
        all_in_names = all_in_names + [partition_name]

    def _body(*args):
        operands = list(args)
        if partition_name is not None:
            operands.append(bass2jax.partition_id_tensor())
        outs = bass2jax._bass_exec_p.bind(
            *operands,
            out_avals=tuple(out_avals),
            in_names=tuple(all_in_names),
            out_names=tuple(out_names),
            lowering_input_output_aliases=(),
            sim_require_finite=True,
            sim_require_nnan=True,
            nc=nc,
        )
        return tuple(outs)

    n_cores = 8
    devices = jax.devices()[:n_cores]
    mesh = Mesh(np.asarray(devices), ("core",))
    spec = PartitionSpec("core")
    sharded = jax.jit(
        shard_map(_body, mesh=mesh,
                  in_specs=(spec,) * (n_params + len(out_names)),
                  out_specs=(spec,) * len(out_names), check_rep=False))
    sharding = NamedSharding(mesh, spec)
    dev_args = [
        jax.device_put(
            np.concatenate([np.asarray(in_maps[c][nm]) for c in range(n_cores)],
                           axis=0), sharding)
        for nm in in_names
    ] + [
        jax.device_put(
            np.zeros((n_cores * z.shape[0], *z.shape[1:]), z.dtype), sharding)
        for z in zero_outs
    ]

    jax.block_until_ready(sharded(*dev_args))

    def run_batch(n):
        t0 = time.perf_counter()
        outs = None
        for _ in range(n):
            outs = sharded(*dev_args)
        jax.block_until_ready(outs)
        return time.perf_counter() - t0

    best = None
    for _ in range(repeats):
        ta = run_batch(n1)
        tb = run_batch(n2)
        marg = (tb - ta) / (n2 - n1)
        best = marg if best is None else min(best, marg)
    return best * 1e9

